# revision 4
# baseline (speedup 1.0000x reference)
"""Causal self-attention (QKV proj + RoPE + causal SDPA + out proj) on 8 trn2 cores.

Sharding: tensor-parallel over heads. Each core owns 2 of 16 heads:
  - Wqkv column-split (the core's q/k/v head rows), Wproj row-split.
  - Each core computes a full-shape partial of the output projection;
    the 8 partials are summed (and transposed back) on the host.

Device-side layout trick: everything runs transposed. The host feeds
x^T [C, B*T]; the QKV projection computes qkv^T = Wslice @ x with the
head dim on partitions, which is exactly what Q@K^T and the output
projection want as inputs, so no on-chip transposes are needed except
V (done with DMA xbar transposes, off the critical engines).

Software pipeline: batch b's projection chunks are interleaved with
batch b-1's attention jobs in emission order, so the PE queue always
has independent filler work while exp/normalization run on the other
engines, and the tensor engine stays continuously busy (keeps the
DVFS pstate at max and hides cross-engine latency).
"""
import sys

sys.path.insert(0, "/opt/trn_rl_repo")

import numpy as np
import ml_dtypes

import concourse.bacc as bacc
import concourse.mybir as mybir
import concourse.tile as tile
from concourse.bass_utils import run_bass_kernel_spmd

N_CORES = 8
C = 2048
H = 16
D = 128
HPC = H // N_CORES          # heads per core = 2
PB = 512                    # row panel width
JB = 128                    # key tile width
NEG = -1.0e30
ROPE_BASE = 10000.0

BF = mybir.dt.bfloat16
F32 = mybir.dt.float32


def build_module(B, T):
    BT = B * T
    CC = C // 128            # contraction chunks for the projection
    FT = 3 * HPC             # qkv f-tiles per core (q0 q1 k0 k1 v0 v1)
    NPB = T // PB            # panels per batch
    NOC = C // 128           # out-proj column tiles
    scale = 1.0 / float(np.sqrt(D))

    nc = bacc.Bacc("TRN2", target_bir_lowering=False, debug=False,
                   num_devices=N_CORES)

    # x pre-tiled on host: xtiles[g, p, cc*PB + r] = x[g*PB + r, cc*128 + p]
    xtiles = nc.dram_tensor("xtiles", [BT // PB, 128, CC * PB], BF,
                            kind="ExternalInput").ap()
    wqkvT = nc.dram_tensor("wqkvT", [C, FT * 128], BF, kind="ExternalInput").ap()
    wprojT = nc.dram_tensor("wprojT", [HPC * 128, C], BF, kind="ExternalInput").ap()
    cosT = nc.dram_tensor("cosT", [128, T], BF, kind="ExternalInput").ap()
    sinT = nc.dram_tensor("sinT", [128, T], F32, kind="ExternalInput").ap()
    maskT = nc.dram_tensor("maskT", [128, 896], F32, kind="ExternalInput").ap()
    permT = nc.dram_tensor("permT", [128, 128], BF, kind="ExternalInput").ap()
    zout = nc.dram_tensor("zout", [C, BT], BF, kind="ExternalOutput").ap()

    with tile.TileContext(nc) as tc:
        with tc.tile_pool(name="sb", bufs=1) as sb, \
             tc.tile_pool(name="ps", bufs=1, space="PSUM") as ps:
            # ---- resident constants ----
            wqkv_sb = sb.tile([128, CC, FT * 128], BF, tag="wqkv", bufs=1)
            nc.sync.dma_start(
                out=wqkv_sb[:],
                in_=wqkvT.rearrange("(cc p) f -> p cc f", p=128))
            wproj_sb = sb.tile([128, HPC, C], BF, tag="wproj", bufs=1)
            nc.sync.dma_start(
                out=wproj_sb[:],
                in_=wprojT.rearrange("(hh p) o -> p hh o", p=128))
            cos_sb = sb.tile([128, T], BF, tag="cos", bufs=1)
            nc.sync.dma_start(out=cos_sb[:], in_=cosT)
            sin_sb = sb.tile([128, T], F32, tag="sin", bufs=1)
            nc.sync.dma_start(out=sin_sb[:], in_=sinT)
            mask_sb = sb.tile([128, 896], F32, tag="mask", bufs=1)
            nc.sync.dma_start(out=mask_sb[:], in_=maskT)
            perm_sb = sb.tile([128, 128], BF, tag="perm", bufs=1)
            nc.sync.dma_start(out=perm_sb[:], in_=permT)
            ones_col = sb.tile([128, 1], BF, tag="ones_c", bufs=1)
            nc.vector.memset(ones_col[:], 1.0)
            ones_row = sb.tile([1, 128], BF, tag="ones_r", bufs=1)
            nc.vector.memset(ones_row[:], 1.0)

            HC = CC // 2
            panels = [(b, pp) for b in range(B) for pp in range(NPB)]

            def load_xt(b, pp):
                g = b * NPB + pp
                xta = sb.tile([128, HC, PB], BF, tag="xta", bufs=3,
                              name=f"xta_{b}_{pp}")
                xtb = sb.tile([128, HC, PB], BF, tag="xtb", bufs=3,
                              name=f"xtb_{b}_{pp}")
                src = xtiles[g].rearrange("p (cc r) -> p cc r", r=PB)
                nc.sync.dma_start(out=xta[:], in_=src[:, :HC, :])
                nc.gpsimd.dma_start(out=xtb[:], in_=src[:, HC:, :])
                return (xta, xtb)

            # global x prefetch state shared across proj generators
            xst = {"q": [], "next": 0}

            def ensure_prefetch(depth=2):
                while (xst["next"] < len(panels)
                       and len(xst["q"]) < depth):
                    xst["q"].append(load_xt(*panels[xst["next"]]))
                    xst["next"] += 1

            # per-batch q/k/v tiles (created during proj emission)
            qkv_tiles = {}
            # round-robin counters for queue balancing
            rr = {"vt": 0, "zst": 0}

            def gen_proj(b):
                """Projection + rope for batch b: one yield per (pp, ft)."""
                q_t = [sb.tile([128, T], BF, tag=f"q{h}", bufs=2,
                               name=f"q{h}_{b}") for h in range(HPC)]
                k_t = [sb.tile([128, T], BF, tag=f"k{h}", bufs=2,
                               name=f"k{h}_{b}") for h in range(HPC)]
                v_t = [sb.tile([128, T // 128, 128], BF, tag=f"v{h}", bufs=2,
                               name=f"v{h}_{b}") for h in range(HPC)]
                qkv_tiles[b] = (q_t, k_t, v_t)
                for pp in range(NPB):
                    ts = slice(pp * PB, pp * PB + PB)
                    ensure_prefetch()
                    xt = xst["q"].pop(0)
                    ensure_prefetch()
                    for ft in range(FT):
                        pps = ps.tile([128, PB], F32, tag="mm", bufs=2)
                        for cc in range(CC):
                            xsrc = xt[0][:, cc, :] if cc < HC \
                                else xt[1][:, cc - HC, :]
                            nc.tensor.matmul(
                                pps[:],
                                lhsT=wqkv_sb[:, cc, ft * 128:(ft + 1) * 128],
                                rhs=xsrc,
                                start=(cc == 0), stop=(cc == CC - 1))
                        if ft < 2 * HPC:   # q or k: apply rope
                            raw = sb.tile([128, PB], BF, tag="qkraw", bufs=2)
                            nc.scalar.copy(out=raw[:], in_=pps[:])
                            rot = ps.tile([128, PB], F32, tag="mm", bufs=2)
                            nc.tensor.matmul(rot[:], lhsT=perm_sb[:],
                                             rhs=raw[:], start=True, stop=True)
                            t1 = sb.tile([128, PB], F32, tag="t1", bufs=2)
                            nc.vector.tensor_mul(out=t1[:], in0=raw[:],
                                                 in1=cos_sb[:, ts])
                            t2 = sb.tile([128, PB], F32, tag="t2", bufs=2)
                            nc.vector.tensor_mul(out=t2[:], in0=rot[:],
                                                 in1=sin_sb[:, ts])
                            dest = (q_t if ft < HPC else k_t)[ft % HPC]
                            nc.vector.tensor_add(out=dest[:, ts], in0=t1[:],
                                                 in1=t2[:])
                        else:              # v: stage + dma-transpose
                            h = ft - 2 * HPC
                            vst = sb.tile([128, PB], BF, tag="vstage", bufs=2)
                            nc.scalar.copy(out=vst[:], in_=pps[:])
                            for q4 in range(PB // 128):
                                jt = pp * (PB // 128) + q4
                                teng = nc.scalar
                                rr["vt"] += 1
                                teng.dma_start_transpose(
                                    out=v_t[h][:, jt, :],
                                    in_=vst[:, q4 * 128:(q4 + 1) * 128])
                        yield

            def gen_attention(a):
                """Attention + out-proj for batch a; yields between units."""
                q_t, k_t, v_t = qkv_tiles.pop(a)
                for pp in range(NPB):
                    nj = (pp + 1) * (PB // JB)
                    q0 = pp * PB
                    ytil = [ps.tile([128, PB], F32, tag="ytil", bufs=2,
                                    name=f"ytil{h}_{a}_{pp}")
                            for h in range(HPC)]
                    denom = [ps.tile([1, PB], F32, tag="small", bufs=2,
                                     name=f"den{h}_{a}_{pp}")
                             for h in range(HPC)]

                    def emit_S(h, j, pp=pp, q0=q0):
                        kk = j - pp * (PB // JB)
                        lo = max(kk, 0) * 128
                        sps = ps.tile([128, PB], F32, tag="sps", bufs=2,
                                      name=f"s{h}_{a}_{pp}_{j}")
                        nc.tensor.matmul(
                            sps[:, lo:PB],
                            lhsT=k_t[h][:, j * JB:(j + 1) * JB],
                            rhs=q_t[h][:, q0 + lo:q0 + PB],
                            start=True, stop=True)
                        return sps

                    def emit_exp(h, j, sps, pp=pp):
                        kk = j - pp * (PB // JB)
                        lo = max(kk, 0) * 128
                        e = sb.tile([128, PB], BF, tag="e", bufs=4,
                                    name=f"e{h}_{a}_{pp}_{j}")
                        if kk >= 0:
                            nc.vector.scalar_tensor_tensor(
                                out=sps[:, lo:lo + 128],
                                in0=sps[:, lo:lo + 128], scalar=scale,
                                in1=mask_sb[:, 384:512],
                                op0=mybir.AluOpType.mult,
                                op1=mybir.AluOpType.add)
                            nc.scalar.activation(
                                out=e[:, lo:lo + 128], in_=sps[:, lo:lo + 128],
                                func=mybir.ActivationFunctionType.Exp)
                            if lo + 128 < PB:
                                nc.scalar.activation(
                                    out=e[:, lo + 128:PB],
                                    in_=sps[:, lo + 128:PB],
                                    func=mybir.ActivationFunctionType.Exp,
                                    scale=scale)
                        else:
                            nc.scalar.activation(
                                out=e[:, lo:PB], in_=sps[:, lo:PB],
                                func=mybir.ActivationFunctionType.Exp,
                                scale=scale)
                        return e

                    def emit_acc(h, j, e, nj=nj, pp=pp):
                        kk = j - pp * (PB // JB)
                        lo = max(kk, 0) * 128
                        nc.tensor.matmul(denom[h][:, lo:PB], lhsT=ones_col[:],
                                         rhs=e[:, lo:PB], start=(j == 0),
                                         stop=(j == nj - 1))
                        nc.tensor.matmul(ytil[h][:, lo:PB],
                                         lhsT=v_t[h][:, j, :],
                                         rhs=e[:, lo:PB], start=(j == 0),
                                         stop=(j == nj - 1))

                    jobs = [(h, j) for j in range(nj) for h in range(HPC)]
                    n = len(jobs)
                    # warmup: S for first two jobs, exp for first job
                    spss = {jobs[0]: emit_S(*jobs[0])}
                    if n > 1:
                        spss[jobs[1]] = emit_S(*jobs[1])
                    es = {jobs[0]: emit_exp(*jobs[0], spss[jobs[0]])}
                    yield
                    for i in range(n):
                        hj = jobs[i]
                        # exp one step ahead of the PE consumer
                        if i + 1 < n:
                            es[jobs[i + 1]] = emit_exp(
                                *jobs[i + 1], spss[jobs[i + 1]])
                        emit_acc(*hj, es.pop(hj))
                        spss.pop(hj)
                        if i + 2 < n:
                            spss[jobs[i + 2]] = emit_S(*jobs[i + 2])
                        yield
                    # ---- normalization ----
                    ypair = []
                    for h in range(HPC):
                        dbf = sb.tile([1, PB], BF, tag="dbf", bufs=2)
                        nc.scalar.copy(out=dbf[:], in_=denom[h][:])
                        bc = ps.tile([128, PB], F32, tag="small", bufs=2,
                                     name=f"bc{h}_{a}_{pp}")
                        nc.tensor.matmul(bc[:], lhsT=ones_row[:],
                                         rhs=dbf[:], start=True, stop=True)
                        rec = sb.tile([128, PB], F32, tag="rec", bufs=2)
                        nc.vector.reciprocal_approx_fast(out=rec[:], in_=bc[:])
                        yp = sb.tile([128, PB], BF, tag="yp", bufs=6)
                        nc.vector.tensor_mul(out=yp[:], in0=ytil[h][:],
                                             in1=rec[:])
                        ypair.append(yp)
                    yield
                    # ---- out-projection for this panel ----
                    r0g = a * T + pp * PB
                    for og in range(0, NOC, 4):
                        for oc in range(og, og + 4):
                            zps = ps.tile([128, PB], F32, tag="small", bufs=2,
                                          name=f"z{a}_{pp}_{oc}")
                            for hh in range(HPC):
                                nc.tensor.matmul(
                                    zps[:],
                                    lhsT=wproj_sb[:, hh,
                                                  oc * 128:(oc + 1) * 128],
                                    rhs=ypair[hh][:],
                                    start=(hh == 0), stop=(hh == HPC - 1))
                            zst = sb.tile([128, PB], BF, tag="zst", bufs=4)
                            if rr["zst"] % 2 == 0:
                                nc.vector.tensor_copy(out=zst[:], in_=zps[:])
                            else:
                                nc.scalar.copy(out=zst[:], in_=zps[:])
                            rr["zst"] += 1
                            nc.gpsimd.dma_start(
                                out=zout[oc * 128:(oc + 1) * 128,
                                         r0g:r0g + PB],
                                in_=zst[:])
                        yield

            def drive(gens):
                """Interleave generators by fractional progress."""
                its = []
                for g, cnt in gens:
                    its.append([g, cnt, 0])
                while its:
                    # pick the least-progressed generator
                    best = min(its, key=lambda it: it[2] / it[1])
                    try:
                        next(best[0])
                        best[2] += 1
                    except StopIteration:
                        its.remove(best)

            def att_units(a):
                tot = 0
                for pp in range(NPB):
                    nj = (pp + 1) * (PB // JB)
                    tot += 1 + nj * HPC + 1 + NOC // 4
                return tot

            for b in range(B + 1):
                gens = []
                if b < B:
                    gens.append((gen_proj(b), NPB * FT))
                if b > 0:
                    gens.append((gen_attention(b - 1), att_units(b - 1)))
                drive(gens)

    nc.compile()
    return nc


_module_cache = {}


def _get_module(B, T):
    key = (B, T)
    if key not in _module_cache:
        _module_cache[key] = build_module(B, T)
    return _module_cache[key]


def _host_prep(x, Wqkv, Wproj, B, T):
    bf16 = ml_dtypes.bfloat16
    BT = B * T
    NP = BT // PB
    CC = C // 128
    x2 = x.reshape(NP, PB, CC, 128)
    xtiles = np.ascontiguousarray(
        x2.transpose(0, 3, 2, 1).reshape(NP, 128, CC * PB)).astype(bf16)

    inv = 1.0 / (ROPE_BASE ** (np.arange(0, D, 2, dtype=np.float32) / D))
    t = np.arange(T, dtype=np.float32)
    fr = np.outer(t, inv)                      # [T, 64]
    emb = np.concatenate([fr, fr], -1)         # [T, 128]
    cosT = np.ascontiguousarray(np.cos(emb).T).astype(bf16)
    sinT = np.ascontiguousarray(np.sin(emb).T).astype(np.float32)

    g = np.arange(896)[None, :]
    p = np.arange(128)[:, None]
    maskT = np.where(g >= p + 384, 0.0, NEG).astype(np.float32)

    permT = np.zeros((128, 128), np.float32)
    for j in range(64):
        permT[j, j + 64] = 1.0                 # rot[i] = q[i-64] for i>=64
    for j in range(64, 128):
        permT[j, j - 64] = -1.0                # rot[i] = -q[i+64] for i<64
    permT = permT.astype(bf16)

    in_maps = []
    for c in range(N_CORES):
        heads = [HPC * c + h for h in range(HPC)]
        rows = []
        for blk in range(3):                   # q, k, v blocks of Wqkv
            for h in heads:
                r0 = blk * C + h * D
                rows.append(Wqkv[r0:r0 + D])
        wslice = np.concatenate(rows, 0)       # [FT*128, C]
        wqkvT = np.ascontiguousarray(wslice.T).astype(bf16)
        cols = np.concatenate([np.arange(h * D, (h + 1) * D) for h in heads])
        wprojT = np.ascontiguousarray(Wproj[:, cols].T).astype(bf16)
        in_maps.append({
            "xtiles": xtiles,
            "wqkvT": wqkvT,
            "wprojT": wprojT,
            "cosT": cosT,
            "sinT": sinT,
            "maskT": maskT,
            "permT": permT,
        })
    return in_maps


last_results = None


def kernel(x, Wqkv, Wproj, _trace=False, _trace_kwargs=None):
    global last_results
    x = np.asarray(x, dtype=np.float32)
    Wqkv = np.asarray(Wqkv, dtype=np.float32)
    Wproj = np.asarray(Wproj, dtype=np.float32)
    B, T, _C = x.shape
    assert _C == C and T % PB == 0

    nc = _get_module(B, T)
    in_maps = _host_prep(x, Wqkv, Wproj, B, T)
    res = run_bass_kernel_spmd(nc, in_maps, core_ids=list(range(N_CORES)),
                               trace=_trace, **(_trace_kwargs or {}))
    last_results = res
    z = res.results[0]["zout"].astype(np.float32)
    for c in range(1, N_CORES):
        z += res.results[c]["zout"].astype(np.float32)
    y = np.ascontiguousarray(z.T).reshape(B, T, C)
    return y


# revision 8
# speedup vs baseline: 1.3734x; 1.3734x over previous
"""Causal self-attention (QKV proj + RoPE + causal SDPA + out proj) on 8 trn2 cores.

Sharding: tensor-parallel over heads. Each core owns 2 of 16 heads:
  - Wqkv column-split (the core's q/k/v head rows), Wproj row-split.
  - Each core computes a full-shape partial of the output projection;
    the 8 partials are summed (and transposed back) on the host.

Device-side layout trick: everything runs transposed. The host feeds
x^T [C, B*T]; the QKV projection computes qkv^T = Wslice @ x with the
head dim on partitions, which is exactly what Q@K^T and the output
projection want as inputs. V is transposed on the PE (transpose-matmul
against an identity) and copied to SBUF by the gpsimd engine.

Software pipeline: batch b's projection chunks are interleaved with
batch b-1's attention jobs in emission order; each panel's output
projection is deferred into the next panel's job stream. The PE queue
always has independent filler work while exp/normalization run on the
other engines, so the tensor engine stays continuously busy.
"""
import sys

sys.path.insert(0, "/opt/trn_rl_repo")

import numpy as np
import ml_dtypes

import concourse.bacc as bacc
import concourse.mybir as mybir
import concourse.tile as tile
from concourse.bass_utils import run_bass_kernel_spmd

N_CORES = 8
C = 2048
H = 16
D = 128
HPC = H // N_CORES          # heads per core = 2
PB = 512                    # row panel width
JB = 128                    # key tile width
NEG = -1.0e30
ROPE_BASE = 10000.0

BF = mybir.dt.bfloat16
F32 = mybir.dt.float32


def build_module(B, T):
    BT = B * T
    CC = C // 128            # contraction chunks for the projection
    FT = 3 * HPC             # qkv f-tiles per core (q0 q1 k0 k1 v0 v1)
    NPB = T // PB            # panels per batch
    NOC = C // 128           # out-proj column tiles
    scale = 1.0 / float(np.sqrt(D))

    nc = bacc.Bacc("TRN2", target_bir_lowering=False, debug=False,
                   num_devices=N_CORES)

    # x pre-tiled on host: xtiles[g, p, cc*PB + r] = x[g*PB + r, cc*128 + p]
    xtiles = nc.dram_tensor("xtiles", [BT // PB, 128, CC * PB], BF,
                            kind="ExternalInput").ap()
    wqkvT = nc.dram_tensor("wqkvT", [C, FT * 128], BF, kind="ExternalInput").ap()
    wprojT = nc.dram_tensor("wprojT", [HPC * 128, C], BF, kind="ExternalInput").ap()
    cosT = nc.dram_tensor("cosT", [128, T], BF, kind="ExternalInput").ap()
    sinT = nc.dram_tensor("sinT", [128, T], F32, kind="ExternalInput").ap()
    maskT = nc.dram_tensor("maskT", [128, 896], F32, kind="ExternalInput").ap()
    permT = nc.dram_tensor("permT", [128, 128], BF, kind="ExternalInput").ap()
    identT = nc.dram_tensor("identT", [128, 128], BF, kind="ExternalInput").ap()
    zout = nc.dram_tensor("zout", [C, BT], BF, kind="ExternalOutput").ap()

    with tile.TileContext(nc) as tc:
        with tc.tile_pool(name="sb", bufs=1) as sb, \
             tc.tile_pool(name="ps", bufs=1, space="PSUM") as ps:
            # ---- resident constants ----
            wqkv_sb = sb.tile([128, CC, FT * 128], BF, tag="wqkv", bufs=1)
            nc.sync.dma_start(
                out=wqkv_sb[:],
                in_=wqkvT.rearrange("(cc p) f -> p cc f", p=128))
            wproj_sb = sb.tile([128, HPC, C], BF, tag="wproj", bufs=1)
            nc.sync.dma_start(
                out=wproj_sb[:],
                in_=wprojT.rearrange("(hh p) o -> p hh o", p=128))
            cos_sb = sb.tile([128, T], BF, tag="cos", bufs=1)
            nc.sync.dma_start(out=cos_sb[:], in_=cosT)
            sin_sb = sb.tile([128, T], F32, tag="sin", bufs=1)
            nc.sync.dma_start(out=sin_sb[:], in_=sinT)
            mask_sb = sb.tile([128, 896], F32, tag="mask", bufs=1)
            nc.sync.dma_start(out=mask_sb[:], in_=maskT)
            perm_sb = sb.tile([128, 128], BF, tag="perm", bufs=1)
            nc.sync.dma_start(out=perm_sb[:], in_=permT)
            ident_sb = sb.tile([128, 128], BF, tag="ident", bufs=1)
            nc.sync.dma_start(out=ident_sb[:], in_=identT)
            ones_col = sb.tile([128, 1], BF, tag="ones_c", bufs=1)
            nc.vector.memset(ones_col[:], 1.0)

            HC = CC // 2
            panels = [(b, pp) for b in range(B) for pp in range(NPB)]

            def load_xt(b, pp):
                g = b * NPB + pp
                xta = sb.tile([128, HC, PB], BF, tag="xta", bufs=4,
                              name=f"xta_{b}_{pp}")
                xtb = sb.tile([128, HC, PB], BF, tag="xtb", bufs=4,
                              name=f"xtb_{b}_{pp}")
                src = xtiles[g].rearrange("p (cc r) -> p cc r", r=PB)
                nc.sync.dma_start(out=xta[:], in_=src[:, :HC, :])
                nc.gpsimd.dma_start(out=xtb[:], in_=src[:, HC:, :])
                return (xta, xtb)

            # global x prefetch state shared across proj generators
            xst = {"q": [], "next": 0}

            def ensure_prefetch(depth=4):
                while (xst["next"] < len(panels)
                       and len(xst["q"]) < depth):
                    xst["q"].append(load_xt(*panels[xst["next"]]))
                    xst["next"] += 1

            qkv_tiles = {}
            rr = {"zst": 0}

            def gen_proj(b):
                """Projection + rope for batch b: one yield per (pp, ft)."""
                q_t = [sb.tile([128, T], BF, tag=f"q{h}", bufs=2,
                               name=f"q{h}_{b}") for h in range(HPC)]
                k_t = [sb.tile([128, T], BF, tag=f"k{h}", bufs=2,
                               name=f"k{h}_{b}") for h in range(HPC)]
                v_t = [sb.tile([128, T // 128, 128], BF, tag=f"v{h}", bufs=2,
                               name=f"v{h}_{b}") for h in range(HPC)]
                qkv_tiles[b] = (q_t, k_t, v_t)
                pvt = {"x": None}

                def flush_vt():
                    if pvt["x"] is None:
                        return
                    vst, h, pp = pvt["x"]
                    pvt["x"] = None
                    for q4 in range(PB // 128):
                        jt = pp * (PB // 128) + q4
                        vtp = ps.tile([128, 128], BF, tag="mm", bufs=2,
                                      name=f"vt{h}_{b}_{pp}_{q4}")
                        nc.tensor.transpose(
                            vtp[:], vst[:, q4 * 128:(q4 + 1) * 128],
                            ident_sb[:])
                        nc.vector.tensor_copy(out=v_t[h][:, jt, :],
                                              in_=vtp[:])

                for pp in range(NPB):
                    ts = slice(pp * PB, pp * PB + PB)
                    ensure_prefetch()
                    xt = xst["q"].pop(0)
                    ensure_prefetch()
                    for ft in range(FT):
                        flush_vt()
                        pps = ps.tile([128, PB], F32, tag="mm", bufs=2)
                        for cc in range(CC):
                            xsrc = xt[0][:, cc, :] if cc < HC \
                                else xt[1][:, cc - HC, :]
                            nc.tensor.matmul(
                                pps[:],
                                lhsT=wqkv_sb[:, cc, ft * 128:(ft + 1) * 128],
                                rhs=xsrc,
                                start=(cc == 0), stop=(cc == CC - 1))
                        if ft < 2 * HPC:   # q or k: apply rope
                            raw = sb.tile([128, PB], BF, tag="qkraw", bufs=2)
                            nc.scalar.copy(out=raw[:], in_=pps[:])
                            rot = ps.tile([128, PB], F32, tag="mm", bufs=2)
                            nc.tensor.matmul(rot[:], lhsT=perm_sb[:],
                                             rhs=raw[:], start=True, stop=True)
                            t1 = sb.tile([128, PB], F32, tag="t1", bufs=2)
                            nc.vector.tensor_mul(out=t1[:], in0=raw[:],
                                                 in1=cos_sb[:, ts])
                            t2 = sb.tile([128, PB], F32, tag="t2", bufs=2)
                            nc.vector.tensor_mul(out=t2[:], in0=rot[:],
                                                 in1=sin_sb[:, ts])
                            dest = (q_t if ft < HPC else k_t)[ft % HPC]
                            nc.vector.tensor_add(out=dest[:, ts], in0=t1[:],
                                                 in1=t2[:])
                        else:              # v: stage, transpose next unit
                            h = ft - 2 * HPC
                            vst = sb.tile([128, PB], BF, tag="vstage", bufs=2)
                            nc.scalar.copy(out=vst[:], in_=pps[:])
                            pvt["x"] = (vst, h, pp)
                        yield
                flush_vt()

            def gen_attention(a):
                """Attention + deferred out-proj for batch a."""
                q_t, k_t, v_t = qkv_tiles.pop(a)
                pending = []       # deferred out-proj og-group closures

                def emit_og(ypair, pp, og):
                    r0g = a * T + pp * PB
                    zstg = sb.tile([128, 4, PB], BF, tag="zst", bufs=3)
                    for i in range(4):
                        oc = og * 4 + i
                        zps = ps.tile([128, PB], F32, tag="mm", bufs=2,
                                      name=f"z{a}_{pp}_{oc}")
                        for hh in range(HPC):
                            nc.tensor.matmul(
                                zps[:],
                                lhsT=wproj_sb[:, hh,
                                              oc * 128:(oc + 1) * 128],
                                rhs=ypair[hh][:],
                                start=(hh == 0), stop=(hh == HPC - 1))
                        if rr["zst"] % 2 == 0:
                            nc.vector.tensor_copy(out=zstg[:, i, :],
                                                  in_=zps[:])
                        else:
                            nc.scalar.copy(out=zstg[:, i, :], in_=zps[:])
                        rr["zst"] += 1
                    dst = zout[og * 512:(og + 1) * 512, r0g:r0g + PB]
                    nc.gpsimd.dma_start(
                        out=dst.rearrange("(i p) c -> p i c", p=128),
                        in_=zstg[:])

                for pp in range(NPB):
                    nj = (pp + 1) * (PB // JB)
                    q0 = pp * PB
                    ytil = [ps.tile([128, PB], F32, tag="ytil", bufs=2,
                                    name=f"ytil{h}_{a}_{pp}")
                            for h in range(HPC)]
                    denom = [ps.tile([1, PB], F32, tag="den", bufs=2,
                                     name=f"den{h}_{a}_{pp}")
                             for h in range(HPC)]

                    def emit_S(h, j, pp=pp, q0=q0):
                        kk = j - pp * (PB // JB)
                        lo = max(kk, 0) * 128
                        sps = ps.tile([128, PB], F32, tag="sps", bufs=2,
                                      name=f"s{h}_{a}_{pp}_{j}")
                        nc.tensor.matmul(
                            sps[:, lo:PB],
                            lhsT=k_t[h][:, j * JB:(j + 1) * JB],
                            rhs=q_t[h][:, q0 + lo:q0 + PB],
                            start=True, stop=True)
                        return sps

                    def emit_exp(h, j, sps, pp=pp):
                        kk = j - pp * (PB // JB)
                        lo = max(kk, 0) * 128
                        e = sb.tile([128, PB], BF, tag="e", bufs=4,
                                    name=f"e{h}_{a}_{pp}_{j}")
                        if kk >= 0:
                            nc.vector.scalar_tensor_tensor(
                                out=sps[:, lo:lo + 128],
                                in0=sps[:, lo:lo + 128], scalar=scale,
                                in1=mask_sb[:, 384:512],
                                op0=mybir.AluOpType.mult,
                                op1=mybir.AluOpType.add)
                            nc.scalar.activation(
                                out=e[:, lo:lo + 128], in_=sps[:, lo:lo + 128],
                                func=mybir.ActivationFunctionType.Exp)
                            if lo + 128 < PB:
                                nc.scalar.activation(
                                    out=e[:, lo + 128:PB],
                                    in_=sps[:, lo + 128:PB],
                                    func=mybir.ActivationFunctionType.Exp,
                                    scale=scale)
                        else:
                            nc.scalar.activation(
                                out=e[:, lo:PB], in_=sps[:, lo:PB],
                                func=mybir.ActivationFunctionType.Exp,
                                scale=scale)
                        return e

                    def emit_acc(h, j, e, nj=nj, pp=pp):
                        kk = j - pp * (PB // JB)
                        lo = max(kk, 0) * 128
                        nc.tensor.matmul(denom[h][:, lo:PB], lhsT=ones_col[:],
                                         rhs=e[:, lo:PB], start=(j == 0),
                                         stop=(j == nj - 1))
                        nc.tensor.matmul(ytil[h][:, lo:PB],
                                         lhsT=v_t[h][:, j, :],
                                         rhs=e[:, lo:PB], start=(j == 0),
                                         stop=(j == nj - 1))

                    jobs = [(h, j) for j in range(nj) for h in range(HPC)]
                    n = len(jobs)
                    spss = {jobs[0]: emit_S(*jobs[0])}
                    if n > 1:
                        spss[jobs[1]] = emit_S(*jobs[1])
                    es = {jobs[0]: emit_exp(*jobs[0], spss[jobs[0]])}
                    yield
                    for i in range(n):
                        hj = jobs[i]
                        if i + 1 < n:
                            es[jobs[i + 1]] = emit_exp(
                                *jobs[i + 1], spss[jobs[i + 1]])
                        emit_acc(*hj, es.pop(hj))
                        spss.pop(hj)
                        if i + 2 < n:
                            spss[jobs[i + 2]] = emit_S(*jobs[i + 2])
                        if pending and i % 2 == 1:
                            pending.pop(0)()
                        yield
                    # ---- PE-free normalization ----
                    ypair = []
                    for h in range(HPC):
                        dbf = sb.tile([1, PB], F32, tag="dbf", bufs=2)
                        nc.scalar.copy(out=dbf[:], in_=denom[h][:])
                        dbc = sb.tile([128, PB], F32, tag="dbc", bufs=2)
                        nc.gpsimd.partition_broadcast(dbc[:], dbf[:],
                                                      channels=128)
                        rec = sb.tile([128, PB], F32, tag="rec", bufs=2)
                        nc.vector.reciprocal_approx_fast(out=rec[:],
                                                         in_=dbc[:])
                        yp = sb.tile([128, PB], BF, tag="yp", bufs=6)
                        nc.vector.tensor_mul(out=yp[:], in0=ytil[h][:],
                                             in1=rec[:])
                        ypair.append(yp)
                    pending = [
                        (lambda ypair=ypair, pp=pp, og=og:
                         emit_og(ypair, pp, og))
                        for og in range(NOC // 4)]
                    yield
                # flush the last panel's out-proj
                while pending:
                    pending.pop(0)()
                    yield

            def drive(gens):
                """Interleave generators by fractional progress."""
                its = []
                for g, cnt in gens:
                    its.append([g, cnt, 0])
                while its:
                    best = min(its, key=lambda it: it[2] / it[1])
                    try:
                        next(best[0])
                        best[2] += 1
                    except StopIteration:
                        its.remove(best)

            def att_units(a):
                tot = NOC // 4     # trailing flush of last panel
                for pp in range(NPB):
                    nj = (pp + 1) * (PB // JB)
                    tot += 1 + nj * HPC + 1
                return tot

            for b in range(B + 1):
                gens = []
                if b < B:
                    gens.append((gen_proj(b), NPB * FT))
                if b > 0:
                    gens.append((gen_attention(b - 1), att_units(b - 1)))
                drive(gens)

    nc.compile()
    return nc


_module_cache = {}


def _get_module(B, T):
    key = (B, T)
    if key not in _module_cache:
        _module_cache[key] = build_module(B, T)
    return _module_cache[key]


def _host_prep(x, Wqkv, Wproj, B, T):
    bf16 = ml_dtypes.bfloat16
    BT = B * T
    NP = BT // PB
    CC = C // 128
    x2 = x.reshape(NP, PB, CC, 128)
    xtiles = np.ascontiguousarray(
        x2.transpose(0, 3, 2, 1).reshape(NP, 128, CC * PB)).astype(bf16)

    inv = 1.0 / (ROPE_BASE ** (np.arange(0, D, 2, dtype=np.float32) / D))
    t = np.arange(T, dtype=np.float32)
    fr = np.outer(t, inv)                      # [T, 64]
    emb = np.concatenate([fr, fr], -1)         # [T, 128]
    cosT = np.ascontiguousarray(np.cos(emb).T).astype(bf16)
    sinT = np.ascontiguousarray(np.sin(emb).T).astype(np.float32)

    g = np.arange(896)[None, :]
    p = np.arange(128)[:, None]
    maskT = np.where(g >= p + 384, 0.0, NEG).astype(np.float32)

    permT = np.zeros((128, 128), np.float32)
    for j in range(64):
        permT[j, j + 64] = 1.0                 # rot[i] = q[i-64] for i>=64
    for j in range(64, 128):
        permT[j, j - 64] = -1.0                # rot[i] = -q[i+64] for i<64
    permT = permT.astype(bf16)
    identT = np.eye(128, dtype=np.float32).astype(bf16)

    in_maps = []
    for c in range(N_CORES):
        heads = [HPC * c + h for h in range(HPC)]
        rows = []
        for blk in range(3):                   # q, k, v blocks of Wqkv
            for h in heads:
                r0 = blk * C + h * D
                rows.append(Wqkv[r0:r0 + D])
        wslice = np.concatenate(rows, 0)       # [FT*128, C]
        wqkvT = np.ascontiguousarray(wslice.T).astype(bf16)
        cols = np.concatenate([np.arange(h * D, (h + 1) * D) for h in heads])
        wprojT = np.ascontiguousarray(Wproj[:, cols].T).astype(bf16)
        in_maps.append({
            "xtiles": xtiles,
            "wqkvT": wqkvT,
            "wprojT": wprojT,
            "cosT": cosT,
            "sinT": sinT,
            "maskT": maskT,
            "permT": permT,
            "identT": identT,
        })
    return in_maps


last_results = None


def kernel(x, Wqkv, Wproj, _trace=False, _trace_kwargs=None):
    global last_results
    x = np.asarray(x, dtype=np.float32)
    Wqkv = np.asarray(Wqkv, dtype=np.float32)
    Wproj = np.asarray(Wproj, dtype=np.float32)
    B, T, _C = x.shape
    assert _C == C and T % PB == 0

    nc = _get_module(B, T)
    in_maps = _host_prep(x, Wqkv, Wproj, B, T)
    res = run_bass_kernel_spmd(nc, in_maps, core_ids=list(range(N_CORES)),
                               trace=_trace, **(_trace_kwargs or {}))
    last_results = res
    z = res.results[0]["zout"].astype(np.float32)
    for c in range(1, N_CORES):
        z += res.results[c]["zout"].astype(np.float32)
    y = np.ascontiguousarray(z.T).reshape(B, T, C)
    return y


# revision 15
# speedup vs baseline: 1.4118x; 1.0279x over previous
"""Causal self-attention (QKV proj + RoPE + causal SDPA + out proj) on 8 trn2 cores.

Sharding: tensor-parallel over heads. Each core owns 2 of 16 heads:
  - Wqkv column-split (the core's q/k/v head rows), Wproj row-split.
  - Each core computes a full-shape partial of the output projection;
    the 8 partials are summed (and transposed back) on the host.

Device-side layout trick: everything runs transposed. The host feeds
x^T [C, B*T]; the QKV projection computes qkv^T = Wslice @ x with the
head dim on partitions, which is exactly what Q@K^T and the output
projection want as inputs. V is transposed on the PE (transpose-matmul
against an identity) and copied to SBUF by the gpsimd engine.

Software pipeline: batch b's projection chunks are interleaved with
batch b-1's attention jobs in emission order; each panel's output
projection is deferred into the next panel's job stream. The PE queue
always has independent filler work while exp/normalization run on the
other engines, so the tensor engine stays continuously busy.
"""
import sys

sys.path.insert(0, "/opt/trn_rl_repo")

import numpy as np
import ml_dtypes

import concourse.bacc as bacc
import concourse.mybir as mybir
import concourse.tile as tile
from concourse.bass_utils import run_bass_kernel_spmd

N_CORES = 8
C = 2048
H = 16
D = 128
HPC = H // N_CORES          # heads per core = 2
PB = 512                    # row panel width
JB = 128                    # key tile width
NEG = -1.0e30
ROPE_BASE = 10000.0

BF = mybir.dt.bfloat16
F32 = mybir.dt.float32


def build_module(B, T):
    BT = B * T
    CC = C // 128            # contraction chunks for the projection
    FT = 3 * HPC             # qkv f-tiles per core (q0 q1 k0 k1 v0 v1)
    NPB = T // PB            # panels per batch
    NOC = C // 128           # out-proj column tiles
    scale = 1.0 / float(np.sqrt(D))

    nc = bacc.Bacc("TRN2", target_bir_lowering=False, debug=False,
                   num_devices=N_CORES)

    # x pre-tiled on host: xtiles[g, p, cc*PB + r] = x[g*PB + r, cc*128 + p]
    xtiles = nc.dram_tensor("xtiles", [BT // PB, 128, CC * PB], BF,
                            kind="ExternalInput").ap()
    wqkvT = nc.dram_tensor("wqkvT", [C, FT * 128], BF, kind="ExternalInput").ap()
    wprojT = nc.dram_tensor("wprojT", [HPC * 128, C], BF, kind="ExternalInput").ap()
    cosT = nc.dram_tensor("cosT", [128, T], BF, kind="ExternalInput").ap()
    sinT = nc.dram_tensor("sinT", [128, T], F32, kind="ExternalInput").ap()
    maskT = nc.dram_tensor("maskT", [128, 896], F32, kind="ExternalInput").ap()
    identT = nc.dram_tensor("identT", [128, 128], BF, kind="ExternalInput").ap()
    zout = nc.dram_tensor("zout", [C, BT], BF, kind="ExternalOutput").ap()

    with tile.TileContext(nc) as tc:
        with tc.tile_pool(name="sb", bufs=1) as sb, \
             tc.tile_pool(name="ps", bufs=1, space="PSUM") as ps:
            # ---- resident constants ----
            wqkv_sb = sb.tile([128, CC, FT * 128], BF, tag="wqkv", bufs=1)
            nc.sync.dma_start(
                out=wqkv_sb[:],
                in_=wqkvT.rearrange("(cc p) f -> p cc f", p=128))
            wproj_sb = sb.tile([128, HPC, C], BF, tag="wproj", bufs=1)
            nc.sync.dma_start(
                out=wproj_sb[:],
                in_=wprojT.rearrange("(hh p) o -> p hh o", p=128))
            cos_sb = sb.tile([128, T], BF, tag="cos", bufs=1)
            nc.sync.dma_start(out=cos_sb[:], in_=cosT)
            sin_sb = sb.tile([128, T], F32, tag="sin", bufs=1)
            nc.sync.dma_start(out=sin_sb[:], in_=sinT)
            mask_sb = sb.tile([128, 896], F32, tag="mask", bufs=1)
            nc.sync.dma_start(out=mask_sb[:], in_=maskT)
            ident_sb = sb.tile([128, 128], BF, tag="ident", bufs=1)
            nc.sync.dma_start(out=ident_sb[:], in_=identT)
            ones_col = sb.tile([128, 1], BF, tag="ones_c", bufs=1)
            nc.vector.memset(ones_col[:], 1.0)

            HC = CC // 2
            panels = [(b, pp) for b in range(B) for pp in range(NPB)]

            def load_xt(b, pp):
                g = b * NPB + pp
                xta = sb.tile([128, HC, PB], BF, tag="xta", bufs=4,
                              name=f"xta_{b}_{pp}")
                xtb = sb.tile([128, HC, PB], BF, tag="xtb", bufs=4,
                              name=f"xtb_{b}_{pp}")
                src = xtiles[g].rearrange("p (cc r) -> p cc r", r=PB)
                nc.sync.dma_start(out=xta[:], in_=src[:, :HC, :])
                nc.gpsimd.dma_start(out=xtb[:], in_=src[:, HC:, :])
                return (xta, xtb)

            # global x prefetch state shared across proj generators
            xst = {"q": [], "next": 0}

            def ensure_prefetch(depth=4):
                while (xst["next"] < len(panels)
                       and len(xst["q"]) < depth):
                    xst["q"].append(load_xt(*panels[xst["next"]]))
                    xst["next"] += 1

            qkv_tiles = {}
            rr = {"zst": 0}

            def gen_proj(b):
                """Projection + rope for batch b: one yield per (pp, ft)."""
                q_t = [sb.tile([128, T], BF, tag=f"q{h}", bufs=2,
                               name=f"q{h}_{b}") for h in range(HPC)]
                k_t = [sb.tile([128, T], BF, tag=f"k{h}", bufs=2,
                               name=f"k{h}_{b}") for h in range(HPC)]
                v_t = [sb.tile([128, T // 128, 128], BF, tag=f"v{h}", bufs=2,
                               name=f"v{h}_{b}") for h in range(HPC)]
                qkv_tiles[b] = (q_t, k_t, v_t)
                pvt = {"x": None}

                def flush_vt():
                    if pvt["x"] is None:
                        return
                    vst, h, pp = pvt["x"]
                    pvt["x"] = None
                    for q4 in range(PB // 128):
                        jt = pp * (PB // 128) + q4
                        vtp = ps.tile([128, 128], BF, tag="mm", bufs=2,
                                      name=f"vt{h}_{b}_{pp}_{q4}")
                        nc.tensor.transpose(
                            vtp[:], vst[:, q4 * 128:(q4 + 1) * 128],
                            ident_sb[:])
                        nc.vector.tensor_copy(out=v_t[h][:, jt, :],
                                              in_=vtp[:])

                for pp in range(NPB):
                    ts = slice(pp * PB, pp * PB + PB)
                    ensure_prefetch()
                    xt = xst["q"].pop(0)
                    ensure_prefetch()
                    for ft in range(FT):
                        flush_vt()
                        pps = ps.tile([128, PB], F32, tag="mm", bufs=2)
                        for cc in range(CC):
                            xsrc = xt[0][:, cc, :] if cc < HC \
                                else xt[1][:, cc - HC, :]
                            nc.tensor.matmul(
                                pps[:],
                                lhsT=wqkv_sb[:, cc, ft * 128:(ft + 1) * 128],
                                rhs=xsrc,
                                start=(cc == 0), stop=(cc == CC - 1))
                        if ft < 2 * HPC:   # q or k: apply rope on DVE
                            # rotate_half via partition-offset reads; the
                            # sign lives in the (pre-negated) sin table
                            t1 = sb.tile([128, PB], F32, tag="t1", bufs=2)
                            nc.vector.tensor_mul(out=t1[:], in0=pps[:],
                                                 in1=cos_sb[:, ts])
                            t2 = sb.tile([128, PB], F32, tag="t2", bufs=2)
                            nc.vector.tensor_mul(out=t2[:64, :],
                                                 in0=pps[64:, :],
                                                 in1=sin_sb[:64, ts])
                            nc.vector.tensor_mul(out=t2[64:, :],
                                                 in0=pps[:64, :],
                                                 in1=sin_sb[64:, ts])
                            dest = (q_t if ft < HPC else k_t)[ft % HPC]
                            nc.vector.tensor_add(out=dest[:, ts], in0=t1[:],
                                                 in1=t2[:])
                        else:              # v: stage, transpose next unit
                            h = ft - 2 * HPC
                            vst = sb.tile([128, PB], BF, tag="vstage", bufs=2)
                            nc.scalar.copy(out=vst[:], in_=pps[:])
                            pvt["x"] = (vst, h, pp)
                        yield
                flush_vt()

            def gen_attention(a):
                """Attention + deferred out-proj for batch a."""
                q_t, k_t, v_t = qkv_tiles.pop(a)
                pending = []       # deferred out-proj og-group closures

                def emit_og(ypair, pp, og):
                    r0g = a * T + pp * PB
                    zstg = sb.tile([128, 4, PB], BF, tag="zst", bufs=3)
                    for i in range(4):
                        oc = og * 4 + i
                        zps = ps.tile([128, PB], F32, tag="mm", bufs=2,
                                      name=f"z{a}_{pp}_{oc}")
                        for hh in range(HPC):
                            nc.tensor.matmul(
                                zps[:],
                                lhsT=wproj_sb[:, hh,
                                              oc * 128:(oc + 1) * 128],
                                rhs=ypair[hh][:],
                                start=(hh == 0), stop=(hh == HPC - 1))
                        if rr["zst"] % 2 == 0:
                            nc.vector.tensor_copy(out=zstg[:, i, :],
                                                  in_=zps[:])
                        else:
                            nc.scalar.copy(out=zstg[:, i, :], in_=zps[:])
                        rr["zst"] += 1
                    dst = zout[og * 512:(og + 1) * 512, r0g:r0g + PB]
                    nc.gpsimd.dma_start(
                        out=dst.rearrange("(i p) c -> p i c", p=128),
                        in_=zstg[:])

                for pp in range(NPB):
                    nj = (pp + 1) * (PB // JB)
                    q0 = pp * PB
                    ytil = [ps.tile([128, PB], F32, tag="ytil", bufs=2,
                                    name=f"ytil{h}_{a}_{pp}")
                            for h in range(HPC)]
                    denom = [ps.tile([1, PB], F32, tag="den", bufs=2,
                                     name=f"den{h}_{a}_{pp}")
                             for h in range(HPC)]

                    def emit_S(h, j, pp=pp, q0=q0):
                        kk = j - pp * (PB // JB)
                        lo = max(kk, 0) * 128
                        sps = ps.tile([128, PB], F32, tag="sps", bufs=2,
                                      name=f"s{h}_{a}_{pp}_{j}")
                        nc.tensor.matmul(
                            sps[:, lo:PB],
                            lhsT=k_t[h][:, j * JB:(j + 1) * JB],
                            rhs=q_t[h][:, q0 + lo:q0 + PB],
                            start=True, stop=True)
                        return sps

                    def emit_exp(h, j, sps, pp=pp):
                        kk = j - pp * (PB // JB)
                        lo = max(kk, 0) * 128
                        e = sb.tile([128, PB], BF, tag="e", bufs=4,
                                    name=f"e{h}_{a}_{pp}_{j}")
                        if kk >= 0:
                            nc.vector.scalar_tensor_tensor(
                                out=sps[:, lo:lo + 128],
                                in0=sps[:, lo:lo + 128], scalar=scale,
                                in1=mask_sb[:, 384:512],
                                op0=mybir.AluOpType.mult,
                                op1=mybir.AluOpType.add)
                            nc.scalar.activation(
                                out=e[:, lo:lo + 128], in_=sps[:, lo:lo + 128],
                                func=mybir.ActivationFunctionType.Exp)
                            if lo + 128 < PB:
                                nc.scalar.activation(
                                    out=e[:, lo + 128:PB],
                                    in_=sps[:, lo + 128:PB],
                                    func=mybir.ActivationFunctionType.Exp,
                                    scale=scale)
                        else:
                            nc.scalar.activation(
                                out=e[:, lo:PB], in_=sps[:, lo:PB],
                                func=mybir.ActivationFunctionType.Exp,
                                scale=scale)
                        return e

                    def emit_acc(h, j, e, nj=nj, pp=pp):
                        kk = j - pp * (PB // JB)
                        lo = max(kk, 0) * 128
                        nc.tensor.matmul(denom[h][:, lo:PB], lhsT=ones_col[:],
                                         rhs=e[:, lo:PB], start=(j == 0),
                                         stop=(j == nj - 1))
                        nc.tensor.matmul(ytil[h][:, lo:PB],
                                         lhsT=v_t[h][:, j, :],
                                         rhs=e[:, lo:PB], start=(j == 0),
                                         stop=(j == nj - 1))

                    jobs = [(h, j) for j in range(nj) for h in range(HPC)]
                    n = len(jobs)
                    spss = {jobs[0]: emit_S(*jobs[0])}
                    if n > 1:
                        spss[jobs[1]] = emit_S(*jobs[1])
                    es = {jobs[0]: emit_exp(*jobs[0], spss[jobs[0]])}
                    yield
                    for i in range(n):
                        hj = jobs[i]
                        if i + 1 < n:
                            es[jobs[i + 1]] = emit_exp(
                                *jobs[i + 1], spss[jobs[i + 1]])
                        emit_acc(*hj, es.pop(hj))
                        spss.pop(hj)
                        if i + 2 < n:
                            spss[jobs[i + 2]] = emit_S(*jobs[i + 2])
                        # deferred out-proj back-loaded so the norm chain
                        # (scalar->gpsimd->vector) has time to finish
                        if pending and i >= n - 2 * len(pending) + 1:
                            pending.pop(0)()
                        yield
                    # ---- PE-free normalization ----
                    ypair = []
                    for h in range(HPC):
                        dbf = sb.tile([1, PB], F32, tag="dbf", bufs=2)
                        nc.scalar.copy(out=dbf[:], in_=denom[h][:])
                        dbc = sb.tile([128, PB], F32, tag="dbc", bufs=2)
                        nc.gpsimd.partition_broadcast(dbc[:], dbf[:],
                                                      channels=128)
                        rec = sb.tile([128, PB], F32, tag="rec", bufs=2)
                        nc.vector.reciprocal_approx_fast(out=rec[:],
                                                         in_=dbc[:])
                        yp = sb.tile([128, PB], BF, tag="yp", bufs=6)
                        nc.vector.tensor_mul(out=yp[:], in0=ytil[h][:],
                                             in1=rec[:])
                        ypair.append(yp)
                    pending = [
                        (lambda ypair=ypair, pp=pp, og=og:
                         emit_og(ypair, pp, og))
                        for og in range(NOC // 4)]
                    yield
                # flush the last panel's out-proj
                while pending:
                    pending.pop(0)()
                    yield

            def drive(gens):
                """Interleave generators by fractional progress."""
                its = []
                for g, cnt in gens:
                    its.append([g, cnt, 0])
                while its:
                    best = min(its, key=lambda it: it[2] / it[1])
                    try:
                        next(best[0])
                        best[2] += 1
                    except StopIteration:
                        its.remove(best)

            def att_units(a):
                tot = NOC // 4     # trailing flush of last panel
                for pp in range(NPB):
                    nj = (pp + 1) * (PB // JB)
                    tot += 1 + nj * HPC + 1
                return tot

            for b in range(B + 1):
                gens = []
                if b < B:
                    gens.append((gen_proj(b), NPB * FT))
                if b > 0:
                    gens.append((gen_attention(b - 1), att_units(b - 1)))
                drive(gens)

    nc.compile()
    return nc


_module_cache = {}


def _get_module(B, T):
    key = (B, T)
    if key not in _module_cache:
        _module_cache[key] = build_module(B, T)
    return _module_cache[key]


def _host_prep(x, Wqkv, Wproj, B, T):
    bf16 = ml_dtypes.bfloat16
    BT = B * T
    NP = BT // PB
    CC = C // 128
    x2 = x.reshape(NP, PB, CC, 128)
    xtiles = np.ascontiguousarray(
        x2.transpose(0, 3, 2, 1).reshape(NP, 128, CC * PB)).astype(bf16)

    inv = 1.0 / (ROPE_BASE ** (np.arange(0, D, 2, dtype=np.float32) / D))
    t = np.arange(T, dtype=np.float32)
    fr = np.outer(t, inv)                      # [T, 64]
    emb = np.concatenate([fr, fr], -1)         # [T, 128]
    cosT = np.ascontiguousarray(np.cos(emb).T).astype(bf16)
    sinT = np.ascontiguousarray(np.sin(emb).T).astype(np.float32)
    sinT[:64] = -sinT[:64]     # rotate_half sign folded into the table

    g = np.arange(896)[None, :]
    p = np.arange(128)[:, None]
    maskT = np.where(g >= p + 384, 0.0, NEG).astype(np.float32)

    identT = np.eye(128, dtype=np.float32).astype(bf16)

    in_maps = []
    for c in range(N_CORES):
        heads = [HPC * c + h for h in range(HPC)]
        rows = []
        for blk in range(3):                   # q, k, v blocks of Wqkv
            for h in heads:
                r0 = blk * C + h * D
                rows.append(Wqkv[r0:r0 + D])
        wslice = np.concatenate(rows, 0)       # [FT*128, C]
        wqkvT = np.ascontiguousarray(wslice.T).astype(bf16)
        cols = np.concatenate([np.arange(h * D, (h + 1) * D) for h in heads])
        wprojT = np.ascontiguousarray(Wproj[:, cols].T).astype(bf16)
        in_maps.append({
            "xtiles": xtiles,
            "wqkvT": wqkvT,
            "wprojT": wprojT,
            "cosT": cosT,
            "sinT": sinT,
            "maskT": maskT,
            "identT": identT,
        })
    return in_maps


last_results = None


def kernel(x, Wqkv, Wproj, _trace=False, _trace_kwargs=None):
    global last_results
    x = np.asarray(x, dtype=np.float32)
    Wqkv = np.asarray(Wqkv, dtype=np.float32)
    Wproj = np.asarray(Wproj, dtype=np.float32)
    B, T, _C = x.shape
    assert _C == C and T % PB == 0

    nc = _get_module(B, T)
    in_maps = _host_prep(x, Wqkv, Wproj, B, T)
    res = run_bass_kernel_spmd(nc, in_maps, core_ids=list(range(N_CORES)),
                               trace=_trace, **(_trace_kwargs or {}))
    last_results = res
    z = res.results[0]["zout"].astype(np.float32)
    for c in range(1, N_CORES):
        z += res.results[c]["zout"].astype(np.float32)
    y = np.ascontiguousarray(z.T).reshape(B, T, C)
    return y


# revision 16
# speedup vs baseline: 1.4405x; 1.0203x over previous
"""Causal self-attention (QKV proj + RoPE + causal SDPA + out proj) on 8 trn2 cores.

Sharding: tensor-parallel over heads. Each core owns 2 of 16 heads:
  - Wqkv column-split (the core's q/k/v head rows), Wproj row-split.
  - Each core computes a full-shape partial of the output projection;
    the 8 partials are summed (and transposed back) on the host.

Device-side layout trick: everything runs transposed. The host feeds
x^T [C, B*T]; the QKV projection computes qkv^T = Wslice @ x with the
head dim on partitions, which is exactly what Q@K^T and the output
projection want as inputs. V is transposed on the PE (transpose-matmul
against an identity) and copied to SBUF by the gpsimd engine.

Software pipeline: batch b's projection chunks are interleaved with
batch b-1's attention jobs in emission order; each panel's output
projection is deferred into the next panel's job stream. The PE queue
always has independent filler work while exp/normalization run on the
other engines, so the tensor engine stays continuously busy.
"""
import sys

sys.path.insert(0, "/opt/trn_rl_repo")

import numpy as np
import ml_dtypes

import concourse.bacc as bacc
import concourse.mybir as mybir
import concourse.tile as tile
from concourse.bass_utils import run_bass_kernel_spmd

N_CORES = 8
C = 2048
H = 16
D = 128
HPC = H // N_CORES          # heads per core = 2
PB = 512                    # row panel width
JB = 128                    # key tile width
NEG = -1.0e30
ROPE_BASE = 10000.0

BF = mybir.dt.bfloat16
F32 = mybir.dt.float32


def build_module(B, T):
    BT = B * T
    CC = C // 128            # contraction chunks for the projection
    FT = 3 * HPC             # qkv f-tiles per core (q0 q1 k0 k1 v0 v1)
    NPB = T // PB            # panels per batch
    NOC = C // 128           # out-proj column tiles
    scale = 1.0 / float(np.sqrt(D))

    nc = bacc.Bacc("TRN2", target_bir_lowering=False, debug=False,
                   num_devices=N_CORES)

    # x pre-tiled on host: xtiles[g, p, cc*PB + r] = x[g*PB + r, cc*128 + p]
    xtiles = nc.dram_tensor("xtiles", [BT // PB, 128, CC * PB], BF,
                            kind="ExternalInput").ap()
    wqkvT = nc.dram_tensor("wqkvT", [C, FT * 128], BF, kind="ExternalInput").ap()
    wprojT = nc.dram_tensor("wprojT", [HPC * 128, C], BF, kind="ExternalInput").ap()
    cosT = nc.dram_tensor("cosT", [128, T], BF, kind="ExternalInput").ap()
    sinT = nc.dram_tensor("sinT", [128, T], F32, kind="ExternalInput").ap()
    maskT = nc.dram_tensor("maskT", [128, 896], F32, kind="ExternalInput").ap()
    identT = nc.dram_tensor("identT", [128, 128], BF, kind="ExternalInput").ap()
    zout = nc.dram_tensor("zout", [C, BT], BF, kind="ExternalOutput").ap()

    with tile.TileContext(nc) as tc:
        with tc.tile_pool(name="sb", bufs=1) as sb, \
             tc.tile_pool(name="ps", bufs=1, space="PSUM") as ps:
            # ---- resident constants ----
            wqkv_sb = sb.tile([128, CC, FT * 128], BF, tag="wqkv", bufs=1)
            nc.sync.dma_start(
                out=wqkv_sb[:],
                in_=wqkvT.rearrange("(cc p) f -> p cc f", p=128))
            wproj_sb = sb.tile([128, HPC, C], BF, tag="wproj", bufs=1)
            nc.sync.dma_start(
                out=wproj_sb[:],
                in_=wprojT.rearrange("(hh p) o -> p hh o", p=128))
            cos_sb = sb.tile([128, T], BF, tag="cos", bufs=1)
            nc.sync.dma_start(out=cos_sb[:], in_=cosT)
            sin_sb = sb.tile([128, T], F32, tag="sin", bufs=1)
            nc.sync.dma_start(out=sin_sb[:], in_=sinT)
            mask_sb = sb.tile([128, 896], F32, tag="mask", bufs=1)
            nc.sync.dma_start(out=mask_sb[:], in_=maskT)
            ident_sb = sb.tile([128, 128], BF, tag="ident", bufs=1)
            nc.sync.dma_start(out=ident_sb[:], in_=identT)
            ones_col = sb.tile([128, 1], BF, tag="ones_c", bufs=1)
            nc.vector.memset(ones_col[:], 1.0)

            HC = CC // 2
            panels = [(b, pp) for b in range(B) for pp in range(NPB)]

            def load_xt(b, pp):
                g = b * NPB + pp
                xta = sb.tile([128, HC, PB], BF, tag="xta", bufs=4,
                              name=f"xta_{b}_{pp}")
                xtb = sb.tile([128, HC, PB], BF, tag="xtb", bufs=4,
                              name=f"xtb_{b}_{pp}")
                src = xtiles[g].rearrange("p (cc r) -> p cc r", r=PB)
                nc.sync.dma_start(out=xta[:], in_=src[:, :HC, :])
                nc.gpsimd.dma_start(out=xtb[:], in_=src[:, HC:, :])
                return (xta, xtb)

            # global x prefetch state shared across proj generators
            xst = {"q": [], "next": 0}

            def ensure_prefetch(depth=4):
                while (xst["next"] < len(panels)
                       and len(xst["q"]) < depth):
                    xst["q"].append(load_xt(*panels[xst["next"]]))
                    xst["next"] += 1

            qkv_tiles = {}
            rr = {"zst": 0}

            def gen_proj(b):
                """Projection + rope for batch b: one yield per (pp, ft)."""
                q_t = [sb.tile([128, T], BF, tag=f"q{h}", bufs=2,
                               name=f"q{h}_{b}") for h in range(HPC)]
                k_t = [sb.tile([128, T], BF, tag=f"k{h}", bufs=2,
                               name=f"k{h}_{b}") for h in range(HPC)]
                v_t = [sb.tile([128, T // 128, 128], BF, tag=f"v{h}", bufs=2,
                               name=f"v{h}_{b}") for h in range(HPC)]
                qkv_tiles[b] = (q_t, k_t, v_t)
                pvt = {"x": None}

                def flush_vt():
                    if pvt["x"] is None:
                        return
                    vst, h, pp = pvt["x"]
                    pvt["x"] = None
                    for q4 in range(PB // 128):
                        jt = pp * (PB // 128) + q4
                        vtp = ps.tile([128, 128], BF, tag="mm", bufs=2,
                                      name=f"vt{h}_{b}_{pp}_{q4}")
                        nc.tensor.transpose(
                            vtp[:], vst[:, q4 * 128:(q4 + 1) * 128],
                            ident_sb[:])
                        nc.vector.tensor_copy(out=v_t[h][:, jt, :],
                                              in_=vtp[:])

                for pp in range(NPB):
                    ts = slice(pp * PB, pp * PB + PB)
                    ensure_prefetch()
                    xt = xst["q"].pop(0)
                    ensure_prefetch()
                    for ft in range(FT):
                        flush_vt()
                        pps = ps.tile([128, PB], F32, tag="mm", bufs=2)
                        for cc in range(CC):
                            xsrc = xt[0][:, cc, :] if cc < HC \
                                else xt[1][:, cc - HC, :]
                            nc.tensor.matmul(
                                pps[:],
                                lhsT=wqkv_sb[:, cc, ft * 128:(ft + 1) * 128],
                                rhs=xsrc,
                                start=(cc == 0), stop=(cc == CC - 1))
                        if ft < 2 * HPC:   # q or k: apply rope on DVE
                            # rotate_half via partition-offset reads; the
                            # sign lives in the (pre-negated) sin table
                            t1 = sb.tile([128, PB], F32, tag="t1", bufs=2)
                            nc.vector.tensor_mul(out=t1[:], in0=pps[:],
                                                 in1=cos_sb[:, ts])
                            t2 = sb.tile([128, PB], F32, tag="t2", bufs=2)
                            nc.vector.tensor_mul(out=t2[:64, :],
                                                 in0=pps[64:, :],
                                                 in1=sin_sb[:64, ts])
                            nc.vector.tensor_mul(out=t2[64:, :],
                                                 in0=pps[:64, :],
                                                 in1=sin_sb[64:, ts])
                            dest = (q_t if ft < HPC else k_t)[ft % HPC]
                            nc.vector.tensor_add(out=dest[:, ts], in0=t1[:],
                                                 in1=t2[:])
                        else:              # v: stage, transpose next unit
                            h = ft - 2 * HPC
                            vst = sb.tile([128, PB], BF, tag="vstage", bufs=2)
                            nc.scalar.copy(out=vst[:], in_=pps[:])
                            pvt["x"] = (vst, h, pp)
                        yield
                flush_vt()

            def gen_attention(a):
                """Attention + deferred out-proj for batch a."""
                q_t, k_t, v_t = qkv_tiles.pop(a)
                pending = []       # deferred out-proj og-group closures

                def emit_og(ypair, pp, og):
                    r0g = a * T + pp * PB
                    zstg = sb.tile([128, 4, PB], BF, tag="zst", bufs=3)
                    for i in range(4):
                        oc = og * 4 + i
                        zps = ps.tile([128, PB], F32, tag="mm", bufs=2,
                                      name=f"z{a}_{pp}_{oc}")
                        for hh in range(HPC):
                            nc.tensor.matmul(
                                zps[:],
                                lhsT=wproj_sb[:, hh,
                                              oc * 128:(oc + 1) * 128],
                                rhs=ypair[hh][:],
                                start=(hh == 0), stop=(hh == HPC - 1))
                        if rr["zst"] % 2 == 0:
                            nc.vector.tensor_copy(out=zstg[:, i, :],
                                                  in_=zps[:])
                        else:
                            nc.scalar.copy(out=zstg[:, i, :], in_=zps[:])
                        rr["zst"] += 1
                    dst = zout[og * 512:(og + 1) * 512, r0g:r0g + PB]
                    nc.gpsimd.dma_start(
                        out=dst.rearrange("(i p) c -> p i c", p=128),
                        in_=zstg[:])

                for pp in range(NPB):
                    nj = (pp + 1) * (PB // JB)
                    q0 = pp * PB
                    ytil = [ps.tile([128, PB], F32, tag="ytil", bufs=2,
                                    name=f"ytil{h}_{a}_{pp}")
                            for h in range(HPC)]
                    denom = [ps.tile([1, PB], F32, tag="den", bufs=2,
                                     name=f"den{h}_{a}_{pp}")
                             for h in range(HPC)]

                    def emit_S(h, j, pp=pp, q0=q0):
                        kk = j - pp * (PB // JB)
                        lo = max(kk, 0) * 128
                        sps = ps.tile([128, PB], F32, tag="sps", bufs=2,
                                      name=f"s{h}_{a}_{pp}_{j}")
                        nc.tensor.matmul(
                            sps[:, lo:PB],
                            lhsT=k_t[h][:, j * JB:(j + 1) * JB],
                            rhs=q_t[h][:, q0 + lo:q0 + PB],
                            start=True, stop=True)
                        return sps

                    def emit_exp(h, j, sps, pp=pp):
                        kk = j - pp * (PB // JB)
                        lo = max(kk, 0) * 128
                        e = sb.tile([128, PB], BF, tag="e", bufs=4,
                                    name=f"e{h}_{a}_{pp}_{j}")
                        if kk >= 0:
                            nc.vector.scalar_tensor_tensor(
                                out=sps[:, lo:lo + 128],
                                in0=sps[:, lo:lo + 128], scalar=scale,
                                in1=mask_sb[:, 384:512],
                                op0=mybir.AluOpType.mult,
                                op1=mybir.AluOpType.add)
                            nc.scalar.activation(
                                out=e[:, lo:lo + 128], in_=sps[:, lo:lo + 128],
                                func=mybir.ActivationFunctionType.Exp)
                            if lo + 128 < PB:
                                nc.scalar.activation(
                                    out=e[:, lo + 128:PB],
                                    in_=sps[:, lo + 128:PB],
                                    func=mybir.ActivationFunctionType.Exp,
                                    scale=scale)
                        else:
                            nc.scalar.activation(
                                out=e[:, lo:PB], in_=sps[:, lo:PB],
                                func=mybir.ActivationFunctionType.Exp,
                                scale=scale)
                        return e

                    def emit_acc(h, j, e, nj=nj, pp=pp):
                        kk = j - pp * (PB // JB)
                        lo = max(kk, 0) * 128
                        nc.tensor.matmul(denom[h][:, lo:PB], lhsT=ones_col[:],
                                         rhs=e[:, lo:PB], start=(j == 0),
                                         stop=(j == nj - 1))
                        nc.tensor.matmul(ytil[h][:, lo:PB],
                                         lhsT=v_t[h][:, j, :],
                                         rhs=e[:, lo:PB], start=(j == 0),
                                         stop=(j == nj - 1))

                    jobs = [(h, j) for j in range(nj) for h in range(HPC)]
                    n = len(jobs)
                    spss = {jobs[0]: emit_S(*jobs[0])}
                    if n > 1:
                        spss[jobs[1]] = emit_S(*jobs[1])
                    es = {jobs[0]: emit_exp(*jobs[0], spss[jobs[0]])}
                    if n > 2:
                        spss[jobs[2]] = emit_S(*jobs[2])
                    if n > 1:
                        es[jobs[1]] = emit_exp(*jobs[1], spss[jobs[1]])
                    yield
                    for i in range(n):
                        hj = jobs[i]
                        if i + 2 < n:
                            es[jobs[i + 2]] = emit_exp(
                                *jobs[i + 2], spss[jobs[i + 2]])
                        emit_acc(*hj, es.pop(hj))
                        spss.pop(hj)
                        if i + 3 < n:
                            spss[jobs[i + 3]] = emit_S(*jobs[i + 3])
                        # deferred out-proj back-loaded so the norm chain
                        # (scalar->gpsimd->vector) has time to finish
                        if pending and i >= n - 2 * len(pending) + 1:
                            pending.pop(0)()
                        yield
                    # ---- PE-free normalization (front half) ----
                    # rec+yp are deferred with the out-proj so they never
                    # block the DVE queue while the broadcast is in flight
                    dbcs = []
                    for h in range(HPC):
                        dbf = sb.tile([1, PB], F32, tag="dbf", bufs=2)
                        nc.scalar.copy(out=dbf[:], in_=denom[h][:])
                        dbc = sb.tile([128, PB], F32, tag="dbc", bufs=2)
                        nc.gpsimd.partition_broadcast(dbc[:], dbf[:],
                                                      channels=128)
                        dbcs.append(dbc)

                    def make_ypair(dbcs=dbcs, ytil=ytil):
                        st = {}

                        def get():
                            if "yp" not in st:
                                ypair = []
                                for h in range(HPC):
                                    rec = sb.tile([128, PB], F32, tag="rec",
                                                  bufs=2)
                                    nc.vector.reciprocal_approx_fast(
                                        out=rec[:], in_=dbcs[h][:])
                                    yp = sb.tile([128, PB], BF, tag="yp",
                                                 bufs=6)
                                    nc.vector.tensor_mul(out=yp[:],
                                                         in0=ytil[h][:],
                                                         in1=rec[:])
                                    ypair.append(yp)
                                st["yp"] = ypair
                            return st["yp"]
                        return get

                    get_ypair = make_ypair()
                    pending = [
                        (lambda get_ypair=get_ypair, pp=pp, og=og:
                         emit_og(get_ypair(), pp, og))
                        for og in range(NOC // 4)]
                    yield
                # flush the last panel's out-proj
                while pending:
                    pending.pop(0)()
                    yield

            def drive(gens):
                """Interleave generators by fractional progress."""
                its = []
                for g, cnt in gens:
                    its.append([g, cnt, 0])
                while its:
                    best = min(its, key=lambda it: it[2] / it[1])
                    try:
                        next(best[0])
                        best[2] += 1
                    except StopIteration:
                        its.remove(best)

            def att_units(a):
                tot = NOC // 4     # trailing flush of last panel
                for pp in range(NPB):
                    nj = (pp + 1) * (PB // JB)
                    tot += 1 + nj * HPC + 1
                return tot

            for b in range(B + 1):
                gens = []
                if b < B:
                    gens.append((gen_proj(b), NPB * FT))
                if b > 0:
                    gens.append((gen_attention(b - 1), att_units(b - 1)))
                drive(gens)

    nc.compile()
    return nc


_module_cache = {}


def _get_module(B, T):
    key = (B, T)
    if key not in _module_cache:
        _module_cache[key] = build_module(B, T)
    return _module_cache[key]


def _host_prep(x, Wqkv, Wproj, B, T):
    bf16 = ml_dtypes.bfloat16
    BT = B * T
    NP = BT // PB
    CC = C // 128
    x2 = x.reshape(NP, PB, CC, 128)
    xtiles = np.ascontiguousarray(
        x2.transpose(0, 3, 2, 1).reshape(NP, 128, CC * PB)).astype(bf16)

    inv = 1.0 / (ROPE_BASE ** (np.arange(0, D, 2, dtype=np.float32) / D))
    t = np.arange(T, dtype=np.float32)
    fr = np.outer(t, inv)                      # [T, 64]
    emb = np.concatenate([fr, fr], -1)         # [T, 128]
    cosT = np.ascontiguousarray(np.cos(emb).T).astype(bf16)
    sinT = np.ascontiguousarray(np.sin(emb).T).astype(np.float32)
    sinT[:64] = -sinT[:64]     # rotate_half sign folded into the table

    g = np.arange(896)[None, :]
    p = np.arange(128)[:, None]
    maskT = np.where(g >= p + 384, 0.0, NEG).astype(np.float32)

    identT = np.eye(128, dtype=np.float32).astype(bf16)

    in_maps = []
    for c in range(N_CORES):
        heads = [HPC * c + h for h in range(HPC)]
        rows = []
        for blk in range(3):                   # q, k, v blocks of Wqkv
            for h in heads:
                r0 = blk * C + h * D
                rows.append(Wqkv[r0:r0 + D])
        wslice = np.concatenate(rows, 0)       # [FT*128, C]
        wqkvT = np.ascontiguousarray(wslice.T).astype(bf16)
        cols = np.concatenate([np.arange(h * D, (h + 1) * D) for h in heads])
        wprojT = np.ascontiguousarray(Wproj[:, cols].T).astype(bf16)
        in_maps.append({
            "xtiles": xtiles,
            "wqkvT": wqkvT,
            "wprojT": wprojT,
            "cosT": cosT,
            "sinT": sinT,
            "maskT": maskT,
            "identT": identT,
        })
    return in_maps


last_results = None


def kernel(x, Wqkv, Wproj, _trace=False, _trace_kwargs=None):
    global last_results
    x = np.asarray(x, dtype=np.float32)
    Wqkv = np.asarray(Wqkv, dtype=np.float32)
    Wproj = np.asarray(Wproj, dtype=np.float32)
    B, T, _C = x.shape
    assert _C == C and T % PB == 0

    nc = _get_module(B, T)
    in_maps = _host_prep(x, Wqkv, Wproj, B, T)
    res = run_bass_kernel_spmd(nc, in_maps, core_ids=list(range(N_CORES)),
                               trace=_trace, **(_trace_kwargs or {}))
    last_results = res
    z = res.results[0]["zout"].astype(np.float32)
    for c in range(1, N_CORES):
        z += res.results[c]["zout"].astype(np.float32)
    y = np.ascontiguousarray(z.T).reshape(B, T, C)
    return y


# revision 22
# speedup vs baseline: 1.4596x; 1.0133x over previous
"""Causal self-attention (QKV proj + RoPE + causal SDPA + out proj) on 8 trn2 cores.

Sharding: tensor-parallel over heads. Each core owns 2 of 16 heads:
  - Wqkv column-split (the core's q/k/v head rows), Wproj row-split.
  - Each core computes a full-shape partial of the output projection;
    the 8 partials are summed (and transposed back) on the host.

Device-side layout trick: everything runs transposed. The host feeds
x^T [C, B*T]; the QKV projection computes qkv^T = Wslice @ x with the
head dim on partitions, which is exactly what Q@K^T and the output
projection want as inputs. V is transposed on the PE (transpose-matmul
against an identity) and copied to SBUF by the gpsimd engine.

Software pipeline: batch b's projection chunks are interleaved with
batch b-1's attention jobs in emission order; each panel's output
projection is deferred into the next panel's job stream. The PE queue
always has independent filler work while exp/normalization run on the
other engines, so the tensor engine stays continuously busy.
"""
import sys

sys.path.insert(0, "/opt/trn_rl_repo")

import numpy as np
import ml_dtypes

import concourse.bacc as bacc
import concourse.mybir as mybir
import concourse.tile as tile
from concourse.bass_utils import run_bass_kernel_spmd

N_CORES = 8
C = 2048
H = 16
D = 128
HPC = H // N_CORES          # heads per core = 2
PB = 512                    # row panel width
JB = 128                    # key tile width
NEG = -1.0e30
ROPE_BASE = 10000.0

BF = mybir.dt.bfloat16
F32 = mybir.dt.float32


def build_module(B, T):
    BT = B * T
    CC = C // 128            # contraction chunks for the projection
    FT = 3 * HPC             # qkv f-tiles per core (q0 q1 k0 k1 v0 v1)
    NPB = T // PB            # panels per batch
    NOC = C // 128           # out-proj column tiles
    scale = 1.0 / float(np.sqrt(D))

    nc = bacc.Bacc("TRN2", target_bir_lowering=False, debug=False,
                   num_devices=N_CORES)

    # x pre-tiled on host: xtiles[g, p, cc*PB + r] = x[g*PB + r, cc*128 + p]
    xtiles = nc.dram_tensor("xtiles", [BT // PB, 128, CC * PB], BF,
                            kind="ExternalInput").ap()
    wqkvT = nc.dram_tensor("wqkvT", [C, FT * 128], BF, kind="ExternalInput").ap()
    wprojT = nc.dram_tensor("wprojT", [HPC * 128, C], BF, kind="ExternalInput").ap()
    cosT = nc.dram_tensor("cosT", [128, T], BF, kind="ExternalInput").ap()
    sinT = nc.dram_tensor("sinT", [128, T], F32, kind="ExternalInput").ap()
    tri01T = nc.dram_tensor("tri01T", [128, 128], BF, kind="ExternalInput").ap()
    identT = nc.dram_tensor("identT", [128, 128], BF, kind="ExternalInput").ap()
    zout = nc.dram_tensor("zout", [C, BT], BF, kind="ExternalOutput").ap()

    with tile.TileContext(nc) as tc:
        with tc.tile_pool(name="sb", bufs=1) as sb, \
             tc.tile_pool(name="ps", bufs=1, space="PSUM") as ps:
            # ---- resident constants ----
            wqkv_sb = sb.tile([128, CC, FT * 128], BF, tag="wqkv", bufs=1)
            nc.sync.dma_start(
                out=wqkv_sb[:],
                in_=wqkvT.rearrange("(cc p) f -> p cc f", p=128))
            wproj_sb = sb.tile([128, HPC, C], BF, tag="wproj", bufs=1)
            nc.sync.dma_start(
                out=wproj_sb[:],
                in_=wprojT.rearrange("(hh p) o -> p hh o", p=128))
            cos_sb = sb.tile([128, T], BF, tag="cos", bufs=1)
            nc.sync.dma_start(out=cos_sb[:], in_=cosT)
            sin_sb = sb.tile([128, T], F32, tag="sin", bufs=1)
            nc.sync.dma_start(out=sin_sb[:], in_=sinT)
            tri_sb = sb.tile([128, 128], BF, tag="tri", bufs=1)
            nc.sync.dma_start(out=tri_sb[:], in_=tri01T)
            ident_sb = sb.tile([128, 128], BF, tag="ident", bufs=1)
            nc.sync.dma_start(out=ident_sb[:], in_=identT)
            ones_col = sb.tile([128, 1], BF, tag="ones_c", bufs=1)
            nc.vector.memset(ones_col[:], 1.0)

            HC = CC // 2
            panels = [(b, pp) for b in range(B) for pp in range(NPB)]

            def load_xt(b, pp):
                g = b * NPB + pp
                xta = sb.tile([128, HC, PB], BF, tag="xta", bufs=4,
                              name=f"xta_{b}_{pp}")
                xtb = sb.tile([128, HC, PB], BF, tag="xtb", bufs=4,
                              name=f"xtb_{b}_{pp}")
                src = xtiles[g].rearrange("p (cc r) -> p cc r", r=PB)
                nc.sync.dma_start(out=xta[:], in_=src[:, :HC, :])
                nc.gpsimd.dma_start(out=xtb[:], in_=src[:, HC:, :])
                return (xta, xtb)

            # global x prefetch state shared across proj generators
            xst = {"q": [], "next": 0}

            def ensure_prefetch(depth=4):
                while (xst["next"] < len(panels)
                       and len(xst["q"]) < depth):
                    xst["q"].append(load_xt(*panels[xst["next"]]))
                    xst["next"] += 1

            qkv_tiles = {}
            rr = {"zst": 0}

            def gen_proj(b):
                """Projection + rope for batch b: one yield per (pp, ft)."""
                q_t = [sb.tile([128, T], BF, tag=f"q{h}", bufs=2,
                               name=f"q{h}_{b}") for h in range(HPC)]
                k_t = [sb.tile([128, T], BF, tag=f"k{h}", bufs=2,
                               name=f"k{h}_{b}") for h in range(HPC)]
                v_t = [sb.tile([128, T // 128, 128], BF, tag=f"v{h}", bufs=2,
                               name=f"v{h}_{b}") for h in range(HPC)]
                qkv_tiles[b] = (q_t, k_t, v_t)
                pvt = {"x": None}

                def flush_vt():
                    if pvt["x"] is None:
                        return
                    vst, h, pp = pvt["x"]
                    pvt["x"] = None
                    for q4 in range(PB // 128):
                        jt = pp * (PB // 128) + q4
                        vtp = ps.tile([128, 128], BF, tag="mm", bufs=2,
                                      name=f"vt{h}_{b}_{pp}_{q4}")
                        nc.tensor.transpose(
                            vtp[:], vst[:, q4 * 128:(q4 + 1) * 128],
                            ident_sb[:])
                        nc.vector.tensor_copy(out=v_t[h][:, jt, :],
                                              in_=vtp[:])

                for pp in range(NPB):
                    ts = slice(pp * PB, pp * PB + PB)
                    ensure_prefetch()
                    xt = xst["q"].pop(0)
                    ensure_prefetch()
                    for ft in range(FT):
                        flush_vt()
                        pps = ps.tile([128, PB], F32, tag="mm", bufs=2)
                        for cc in range(CC):
                            xsrc = xt[0][:, cc, :] if cc < HC \
                                else xt[1][:, cc - HC, :]
                            nc.tensor.matmul(
                                pps[:],
                                lhsT=wqkv_sb[:, cc, ft * 128:(ft + 1) * 128],
                                rhs=xsrc,
                                start=(cc == 0), stop=(cc == CC - 1))
                        if ft < 2 * HPC:   # q or k: apply rope on DVE
                            # rotate_half via partition-offset reads; the
                            # sign lives in the (pre-negated) sin table
                            t1 = sb.tile([128, PB], F32, tag="t1", bufs=2)
                            nc.vector.tensor_mul(out=t1[:], in0=pps[:],
                                                 in1=cos_sb[:, ts])
                            t2 = sb.tile([128, PB], F32, tag="t2", bufs=2)
                            nc.vector.tensor_mul(out=t2[:64, :],
                                                 in0=pps[64:, :],
                                                 in1=sin_sb[:64, ts])
                            nc.vector.tensor_mul(out=t2[64:, :],
                                                 in0=pps[:64, :],
                                                 in1=sin_sb[64:, ts])
                            dest = (q_t if ft < HPC else k_t)[ft % HPC]
                            nc.vector.tensor_add(out=dest[:, ts], in0=t1[:],
                                                 in1=t2[:])
                        else:              # v: stage, transpose next unit
                            h = ft - 2 * HPC
                            vst = sb.tile([128, PB], BF, tag="vstage", bufs=2)
                            nc.scalar.copy(out=vst[:], in_=pps[:])
                            pvt["x"] = (vst, h, pp)
                        yield
                flush_vt()

            def gen_attention(a):
                """Attention + deferred out-proj for batch a."""
                q_t, k_t, v_t = qkv_tiles.pop(a)
                pending = []       # deferred out-proj og-group closures

                def emit_og(ypair, pp, og):
                    r0g = a * T + pp * PB
                    zstg = sb.tile([128, 4, PB], BF, tag="zst", bufs=3)
                    for i in range(4):
                        oc = og * 4 + i
                        zps = ps.tile([128, PB], F32, tag="mm", bufs=2,
                                      name=f"z{a}_{pp}_{oc}")
                        for hh in range(HPC):
                            nc.tensor.matmul(
                                zps[:],
                                lhsT=wproj_sb[:, hh,
                                              oc * 128:(oc + 1) * 128],
                                rhs=ypair[hh][:],
                                start=(hh == 0), stop=(hh == HPC - 1))
                        if rr["zst"] % 2 == 0:
                            nc.vector.tensor_copy(out=zstg[:, i, :],
                                                  in_=zps[:])
                        else:
                            nc.scalar.copy(out=zstg[:, i, :], in_=zps[:])
                        rr["zst"] += 1
                    dst = zout[og * 512:(og + 1) * 512, r0g:r0g + PB]
                    nc.gpsimd.dma_start(
                        out=dst.rearrange("(i p) c -> p i c", p=128),
                        in_=zstg[:])

                for pp in range(NPB):
                    nj = (pp + 1) * (PB // JB)
                    q0 = pp * PB
                    ytil = [ps.tile([128, PB], F32, tag="ytil", bufs=2,
                                    name=f"ytil{h}_{a}_{pp}")
                            for h in range(HPC)]
                    denom = [ps.tile([1, PB], F32, tag="den", bufs=2,
                                     name=f"den{h}_{a}_{pp}")
                             for h in range(HPC)]

                    def emit_S(h, j, pp=pp, q0=q0):
                        kk = j - pp * (PB // JB)
                        lo = max(kk, 0) * 128
                        sps = ps.tile([128, PB], F32, tag="sps", bufs=2,
                                      name=f"s{h}_{a}_{pp}_{j}")
                        nc.tensor.matmul(
                            sps[:, lo:PB],
                            lhsT=k_t[h][:, j * JB:(j + 1) * JB],
                            rhs=q_t[h][:, q0 + lo:q0 + PB],
                            start=True, stop=True)
                        return sps

                    def emit_exp(h, j, sps, pp=pp):
                        kk = j - pp * (PB // JB)
                        lo = max(kk, 0) * 128
                        e = sb.tile([128, PB], BF, tag="e", bufs=6,
                                    name=f"e{h}_{a}_{pp}_{j}")
                        nc.scalar.activation(
                            out=e[:, lo:PB], in_=sps[:, lo:PB],
                            func=mybir.ActivationFunctionType.Exp,
                            scale=scale)
                        if kk >= 0:
                            # zero the below-diagonal triangle of this block
                            nc.vector.tensor_mul(
                                out=e[:, lo:lo + 128], in0=e[:, lo:lo + 128],
                                in1=tri_sb[:])
                        return e

                    def emit_acc(h, j, e, nj=nj, pp=pp):
                        kk = j - pp * (PB // JB)
                        lo = max(kk, 0) * 128
                        nc.tensor.matmul(denom[h][:, lo:PB], lhsT=ones_col[:],
                                         rhs=e[:, lo:PB], start=(j == 0),
                                         stop=(j == nj - 1))
                        nc.tensor.matmul(ytil[h][:, lo:PB],
                                         lhsT=v_t[h][:, j, :],
                                         rhs=e[:, lo:PB], start=(j == 0),
                                         stop=(j == nj - 1))

                    jobs = [(h, j) for j in range(nj) for h in range(HPC)]
                    n = len(jobs)
                    L = 3              # exp lead over the PE consumer
                    spss = {jobs[0]: emit_S(*jobs[0])}
                    if n > 1:
                        spss[jobs[1]] = emit_S(*jobs[1])
                    es = {}
                    for t in range(min(L, n)):
                        es[jobs[t]] = emit_exp(*jobs[t], spss[jobs[t]])
                        if t + 2 < n:
                            spss[jobs[t + 2]] = emit_S(*jobs[t + 2])
                    yield
                    for i in range(n):
                        hj = jobs[i]
                        if i + L < n:
                            es[jobs[i + L]] = emit_exp(
                                *jobs[i + L], spss[jobs[i + L]])
                        emit_acc(*hj, es.pop(hj))
                        spss.pop(hj)
                        if i + L + 2 < n:
                            spss[jobs[i + L + 2]] = emit_S(*jobs[i + L + 2])
                        # deferred out-proj back-loaded so the norm chain
                        # (scalar->gpsimd->vector) has time to finish
                        if pending and i >= n - 2 * len(pending) + 1:
                            pending.pop(0)()
                        yield
                    # ---- PE-free normalization (front half) ----
                    # rec+yp are deferred with the out-proj so they never
                    # block the DVE queue while the broadcast is in flight
                    dbcs = []
                    for h in range(HPC):
                        dbf = sb.tile([1, PB], F32, tag="dbf", bufs=2)
                        nc.scalar.copy(out=dbf[:], in_=denom[h][:])
                        dbc = sb.tile([128, PB], F32, tag="dbc", bufs=2)
                        nc.gpsimd.partition_broadcast(dbc[:], dbf[:],
                                                      channels=128)
                        dbcs.append(dbc)

                    def make_ypair(dbcs=dbcs, ytil=ytil):
                        st = {}

                        def get():
                            if "yp" not in st:
                                ypair = []
                                for h in range(HPC):
                                    rec = sb.tile([128, PB], F32, tag="rec",
                                                  bufs=2)
                                    nc.vector.reciprocal_approx_fast(
                                        out=rec[:], in_=dbcs[h][:])
                                    yp = sb.tile([128, PB], BF, tag="yp",
                                                 bufs=6)
                                    nc.vector.tensor_mul(out=yp[:],
                                                         in0=ytil[h][:],
                                                         in1=rec[:])
                                    ypair.append(yp)
                                st["yp"] = ypair
                            return st["yp"]
                        return get

                    get_ypair = make_ypair()
                    pending = [
                        (lambda get_ypair=get_ypair, pp=pp, og=og:
                         emit_og(get_ypair(), pp, og))
                        for og in range(NOC // 4)]
                    yield
                # flush the last panel's out-proj
                while pending:
                    pending.pop(0)()
                    yield

            def drive(gens):
                """Interleave generators by fractional progress."""
                its = []
                for g, cnt in gens:
                    its.append([g, cnt, 0])
                while its:
                    best = min(its, key=lambda it: it[2] / it[1])
                    try:
                        next(best[0])
                        best[2] += 1
                    except StopIteration:
                        its.remove(best)

            def att_units(a):
                tot = NOC // 4     # trailing flush of last panel
                for pp in range(NPB):
                    nj = (pp + 1) * (PB // JB)
                    tot += 1 + nj * HPC + 1
                return tot

            for b in range(B + 1):
                gens = []
                if b < B:
                    gens.append((gen_proj(b), NPB * FT))
                if b > 0:
                    gens.append((gen_attention(b - 1), att_units(b - 1)))
                drive(gens)

    nc.compile()
    return nc


_module_cache = {}


def _get_module(B, T):
    key = (B, T)
    if key not in _module_cache:
        _module_cache[key] = build_module(B, T)
    return _module_cache[key]


def _host_prep(x, Wqkv, Wproj, B, T):
    bf16 = ml_dtypes.bfloat16
    BT = B * T
    NP = BT // PB
    CC = C // 128
    x2 = x.reshape(NP, PB, CC, 128)
    xtiles = np.ascontiguousarray(
        x2.transpose(0, 3, 2, 1).reshape(NP, 128, CC * PB)).astype(bf16)

    inv = 1.0 / (ROPE_BASE ** (np.arange(0, D, 2, dtype=np.float32) / D))
    t = np.arange(T, dtype=np.float32)
    fr = np.outer(t, inv)                      # [T, 64]
    emb = np.concatenate([fr, fr], -1)         # [T, 128]
    cosT = np.ascontiguousarray(np.cos(emb).T).astype(bf16)
    sinT = np.ascontiguousarray(np.sin(emb).T).astype(np.float32)
    sinT[:64] = -sinT[:64]     # rotate_half sign folded into the table

    g = np.arange(128)[None, :]
    p = np.arange(128)[:, None]
    tri01T = (g >= p).astype(np.float32).astype(bf16)

    identT = np.eye(128, dtype=np.float32).astype(bf16)

    in_maps = []
    for c in range(N_CORES):
        heads = [HPC * c + h for h in range(HPC)]
        rows = []
        for blk in range(3):                   # q, k, v blocks of Wqkv
            for h in heads:
                r0 = blk * C + h * D
                rows.append(Wqkv[r0:r0 + D])
        wslice = np.concatenate(rows, 0)       # [FT*128, C]
        wqkvT = np.ascontiguousarray(wslice.T).astype(bf16)
        cols = np.concatenate([np.arange(h * D, (h + 1) * D) for h in heads])
        wprojT = np.ascontiguousarray(Wproj[:, cols].T).astype(bf16)
        in_maps.append({
            "xtiles": xtiles,
            "wqkvT": wqkvT,
            "wprojT": wprojT,
            "cosT": cosT,
            "sinT": sinT,
            "tri01T": tri01T,
            "identT": identT,
        })
    return in_maps


last_results = None


def kernel(x, Wqkv, Wproj, _trace=False, _trace_kwargs=None):
    global last_results
    x = np.asarray(x, dtype=np.float32)
    Wqkv = np.asarray(Wqkv, dtype=np.float32)
    Wproj = np.asarray(Wproj, dtype=np.float32)
    B, T, _C = x.shape
    assert _C == C and T % PB == 0

    nc = _get_module(B, T)
    in_maps = _host_prep(x, Wqkv, Wproj, B, T)
    res = run_bass_kernel_spmd(nc, in_maps, core_ids=list(range(N_CORES)),
                               trace=_trace, **(_trace_kwargs or {}))
    last_results = res
    z = res.results[0]["zout"].astype(np.float32)
    for c in range(1, N_CORES):
        z += res.results[c]["zout"].astype(np.float32)
    y = np.ascontiguousarray(z.T).reshape(B, T, C)
    return y
